# revision 15
# baseline (speedup 1.0000x reference)
"""CrossScaleAttention Trainium2 kernel.

Full (unsharded) contract: kernel(query, key, value) with shapes
  query/key/value: (4, 4096, 256) float32  ->  out (4, 4096, 256) float32

reference math:
  q = l2norm(query); k = l2norm(key)
  out = softmax((q @ k^T) * 32**-0.5) @ value

Sharding: 8 cores; core c computes batch c//2, query rows (c%2)*2048..+2048,
with that batch's full K/V resident per core (no collectives).

Per-core algorithm (all matmuls contract over the partition dim):
  - K is transposed RAW into K^T [d, keys] (PE transpose); its row norm is
    applied later as the exp's per-partition scale, so K^T production only
    depends on the DMA. Q is scaled by SCALE/||q|| before its transpose
    (a per-query scale cannot ride the exp, whose scale is per-partition).
  - row norms: sum-of-squares on DVE (one mul+reduce pair per tile group),
    rsqrt on DVE via the 0x5f3759df bit trick + 2 Newton steps (no ACT
    table switches at all -- ACT runs a single Exp table the whole kernel).
  - S^T chunks [128 keys, 512 queries] on PE (K^T chunk stationary);
    exp(scale_k * s) via ACT straight PSUM->SBUF producing P^T. No
    max-subtraction needed: scores are cosine sims * 0.177.
  - AV: out_psum[128 q, 258] += P^T_chunk.T @ [V | 1 1]; the ones columns
    accumulate the softmax denominator inside the same matmul chain
    (padded to 258: fp32r matmuls need an even moving-dim).
  - epilogue: out = out_psum[:, :256] * (1 / out_psum[:, 256]).
  - inputs arrive via a few large multi-tile DMAs (one InstDMACopy spreads
    across all 16 SDMA slots), ordered so block 0's operands land first.
"""

import sys

if "/opt/trn_rl_repo" not in sys.path:
    sys.path.insert(0, "/opt/trn_rl_repo")

import numpy as np

import concourse.bass as bass
import concourse.mybir as mybir
import concourse.tile as tile
from concourse import bacc
from concourse.bass_utils import run_bass_kernel_spmd
from concourse.masks import make_identity

F32 = mybir.dt.float32
F32R = mybir.dt.float32r
I32 = mybir.dt.int32

B, NQ_FULL, NK, D = 4, 4096, 4096, 256
N_CORES = 8
NQ = NQ_FULL * B // N_CORES  # 2048 queries per core
P = 128
DC = D // P          # 2 d-chunks
KC = NK // P         # 32 key chunks
QTI = NQ // P        # 16 q tiles
QB = 512             # queries per block
NB = NQ // QB        # 4 blocks
QT = QB // P         # 4 q-subtiles per block
VW = D + 2           # V columns padded with two 1.0 columns (even moving-dim)
NT = KC + QTI        # 48 row tiles total
SCALE = float(D // 8) ** -0.5  # head_dim**-0.5 = 32**-0.5
RSQRT_MAGIC = 0x5F3759DF

Exp = mybir.ActivationFunctionType.Exp

# natall/ssall/rinv_all positions: q0-3 -> 0..3, k0-31 -> 4..35, q4-15 -> 36..47
QPOS0, KPOS, QPOS1 = 0, 4, 36


def _build_program():
    nc = bacc.Bacc(
        "TRN2",
        target_bir_lowering=False,
        debug=False,
        enable_asserts=False,
        num_devices=N_CORES,
    )
    q_d = nc.dram_tensor("q", (NQ, D), F32, kind="ExternalInput").ap()
    k_d = nc.dram_tensor("k", (NK, D), F32, kind="ExternalInput").ap()
    v_d = nc.dram_tensor("v", (NK, D), F32, kind="ExternalInput").ap()
    o_d = nc.dram_tensor("o", (NQ, D), F32, kind="ExternalOutput").ap()

    k_re = k_d.rearrange("(i p) d -> p i d", p=P)  # [128, 32, 256]
    q_re = q_d.rearrange("(i p) d -> p i d", p=P)  # [128, 16, 256]
    v_re = v_d.rearrange("(i p) d -> p i d", p=P)  # [128, 32, 256]

    with tile.TileContext(nc) as tc:
        with (
            tc.tile_pool(name="const", bufs=1) as const_pool,
            tc.tile_pool(name="persist", bufs=1) as persist,
            tc.tile_pool(name="stage", bufs=2) as stage,
            tc.tile_pool(name="loads", bufs=4) as loads,
            tc.tile_pool(name="small", bufs=8) as small,
            tc.tile_pool(name="pt", bufs=4) as pt_pool,
            tc.tile_pool(name="outs", bufs=3) as out_pool,
            tc.tile_pool(name="ps", bufs=4, space="PSUM") as ps_pool,
            tc.tile_pool(name="avps", bufs=1, space="PSUM") as av_pool,
        ):
            ident = const_pool.tile([P, P], F32)
            make_identity(nc, ident)
            ones = const_pool.tile([P, 1], F32)
            nc.vector.memset(ones, 1.0)
            magic = const_pool.tile([P, 1], I32)
            nc.vector.memset(magic, RSQRT_MAGIC)

            # persistent operands
            kt = persist.tile([P, DC, NK], F32R)    # K^T: [d, keys] (RAW rows)
            qt = persist.tile([P, DC, NQ], F32R)    # Q^T: [d, queries] scaled
            va = persist.tile([P, KC, VW], F32R)    # [keys, d | ones ones]
            natall = persist.tile([P, NT, D], F32)  # raw rows
            ssall = persist.tile([P, NT], F32)      # row sum-of-squares
            rinv_all = persist.tile([P, NT], F32)   # (pre)scale / ||row||

            nc.vector.tensor_copy(
                va[:, :, D:VW], ones[:, :, None].to_broadcast((P, KC, 2))
            )

            # ---- input DMAs: a few multi-tile transfers, block-0 deps first
            nc.sync.dma_start(natall[:, 0:4, :], q_re[:, 0:4, :])      # q0-3
            nc.sync.dma_start(natall[:, 4:12, :], k_re[:, 0:8, :])     # k0-7
            vstg = []
            for g, (v0, v1) in enumerate(((0, 8), (8, 16), (16, 24), (24, KC))):
                vs = stage.tile([P, 8, D], F32, tag="vl", name=f"vs{g}")
                vstg.append((vs, v0, v1))
            nc.sync.dma_start(vstg[0][0], v_re[:, 0:8, :])
            nc.sync.dma_start(vstg[1][0], v_re[:, 8:16, :])
            nc.sync.dma_start(natall[:, 12:20, :], k_re[:, 8:16, :])   # k8-15
            nc.sync.dma_start(vstg[2][0], v_re[:, 16:24, :])
            nc.sync.dma_start(natall[:, 20:36, :], k_re[:, 16:KC, :])  # k16-31
            nc.sync.dma_start(vstg[3][0], v_re[:, 24:KC, :])
            nc.sync.dma_start(natall[:, 36:48, :], q_re[:, 4:QTI, :])  # q4-15

            # V copies (f32 -> f32r round): first chunk on idle ACT; the rest
            # in small per-2-tile pieces the DVE scheduler can interleave,
            # emitted below at their DMA-arrival points.
            nc.scalar.copy(va[:, 0:8, :D], vstg[0][0])

            def v_copies(g):
                vs, v0, v1 = vstg[g]
                for j in range(0, v1 - v0, 2):
                    nc.vector.tensor_copy(
                        va[:, v0 + j : v0 + j + 2, :D], vs[:, j : j + 2, :]
                    )

            # ---- row norms (all DVE; no ACT tables involved) ----
            def norms(lo, hi, q_scale):
                n = hi - lo
                sq = stage.tile([P, n, D], F32, tag="sqg", name=f"sqg{lo}")
                nat = natall[:, lo:hi, :]
                nc.vector.tensor_mul(sq, nat, nat)
                ss = ssall[:, lo:hi]
                nc.vector.tensor_reduce(
                    ss, sq, axis=mybir.AxisListType.X, op=mybir.AluOpType.add
                )
                # rsqrt: bit-trick seed + 2 Newton iterations (err ~5e-6)
                y = rinv_all[:, lo:hi]
                yi = y.bitcast(I32)
                nc.vector.tensor_scalar(
                    yi, ss.bitcast(I32), 1, None,
                    op0=mybir.AluOpType.logical_shift_right,
                )
                nc.vector.tensor_tensor(
                    yi, magic.to_broadcast((P, n)), yi, mybir.AluOpType.subtract
                )
                t = small.tile([P, n], F32, tag="nt", name=f"nt{lo}")
                for _ in range(2):
                    nc.vector.tensor_mul(t, y, y)
                    nc.vector.tensor_mul(t, t, ss)
                    nc.vector.tensor_scalar(
                        t, t, -0.5, 1.5,
                        op0=mybir.AluOpType.mult, op1=mybir.AluOpType.add,
                    )
                    nc.vector.tensor_mul(y, y, t)
                if q_scale:
                    nc.vector.tensor_scalar_mul(y, y, SCALE)

            def finish(pos, kind, idx, copy_eng):
                """PE-transpose row-tile `pos` into kt/qt column idx."""
                if kind == "q":
                    src = loads.tile([P, D], F32, tag="xn", name=f"xn{pos}")
                    nc.vector.tensor_scalar_mul(
                        src, natall[:, pos, :], rinv_all[:, pos : pos + 1]
                    )
                    dst = qt
                else:
                    src = natall[:, pos, :]
                    dst = kt
                tps = ps_pool.tile([P, QB], F32, tag="st", name=f"tp{pos}")
                for dc in range(DC):
                    nc.tensor.transpose(
                        tps[:, dc * P : (dc + 1) * P],
                        src[:, dc * P : (dc + 1) * P],
                        ident,
                    )
                csrc = tps[:, :D].rearrange("p (c n) -> p c n", c=DC)
                cdst = dst[:, :, idx * P : (idx + 1) * P]
                if copy_eng == "scalar":
                    nc.scalar.copy(cdst, csrc)
                else:
                    nc.vector.tensor_copy(cdst, csrc)

            norms(0, 4, True)                       # q0-3
            for i in range(4):
                finish(QPOS0 + i, "q", i, "scalar")
            norms(4, 12, False)                     # k0-7
            for i in range(8):
                finish(KPOS + i, "k", i, "scalar")
            v_copies(1)                             # v8-15
            norms(12, 20, False)                    # k8-15
            for i in range(8, 16):
                finish(KPOS + i, "k", i, "scalar")
            v_copies(2)                             # v16-23
            norms(20, 28, False)                    # k16-23
            for i in range(16, 24):
                finish(KPOS + i, "k", i, "vector")
            norms(28, 36, False)                    # k24-31
            v_copies(3)                             # v24-31
            for i in range(24, KC):
                finish(KPOS + i, "k", i, "vector")
            norms(36, 48, True)                     # q4-15
            for i in range(4, QTI):
                finish(QPOS1 + i - 4, "q", i, "vector")

            # ---- main loop ----
            for blk in range(NB):
                avs = [
                    av_pool.tile([P, VW], F32, tag=f"av{t}", name=f"av{t}_{blk}")
                    for t in range(QT)
                ]
                for kk in range(KC):
                    st = ps_pool.tile([P, QB], F32, tag="st", name=f"st{blk}_{kk}")
                    for dc in range(DC):
                        nc.tensor.matmul(
                            st,
                            lhsT=kt[:, dc, kk * P : (kk + 1) * P],
                            rhs=qt[:, dc, blk * QB : (blk + 1) * QB],
                            start=(dc == 0),
                            stop=(dc == DC - 1),
                        )
                    pt = pt_pool.tile([P, QB], F32R, tag="pt", name=f"pt{blk}_{kk}")
                    nc.scalar.activation(
                        pt, st, Exp, scale=rinv_all[:, KPOS + kk : KPOS + kk + 1]
                    )
                    for t in range(QT):
                        nc.tensor.matmul(
                            avs[t],
                            lhsT=pt[:, t * P : (t + 1) * P],
                            rhs=va[:, kk, :],
                            start=(kk == 0),
                            stop=(kk == KC - 1),
                        )
                for t in range(QT):
                    rec = small.tile([P, 1], F32, tag="rec")
                    nc.vector.reciprocal(rec, avs[t][:, D : D + 1])
                    ot = out_pool.tile([P, D], F32, tag="ot")
                    nc.vector.tensor_scalar_mul(ot, avs[t][:, :D], rec)
                    row = blk * QB + t * P
                    nc.sync.dma_start(o_d[row : row + P, :], ot)

    nc.compile()
    return nc


_CACHED = {}


def _get_program():
    if "nc" not in _CACHED:
        _CACHED["nc"] = _build_program()
    return _CACHED["nc"]


def _make_in_maps(query, key, value):
    in_maps = []
    for c in range(N_CORES):
        b = c // (N_CORES // B)
        qs = (c % (N_CORES // B)) * NQ
        in_maps.append(
            {
                "q": np.ascontiguousarray(query[b, qs : qs + NQ], dtype=np.float32),
                "k": np.ascontiguousarray(key[b], dtype=np.float32),
                "v": np.ascontiguousarray(value[b], dtype=np.float32),
            }
        )
    return in_maps


def run_sharded(query, key, value, trace=False):
    """Returns (out, BassKernelResults)."""
    nc = _get_program()
    in_maps = _make_in_maps(query, key, value)
    res = run_bass_kernel_spmd(nc, in_maps, core_ids=list(range(N_CORES)), trace=trace)
    out = np.empty((B, NQ_FULL, D), dtype=np.float32)
    for c in range(N_CORES):
        b = c // (N_CORES // B)
        qs = (c % (N_CORES // B)) * NQ
        out[b, qs : qs + NQ] = res.results[c]["o"]
    return out, res


def kernel(query, key, value):
    query = np.asarray(query)
    key = np.asarray(key)
    value = np.asarray(value)
    out, _ = run_sharded(query, key, value)
    return out


# revision 16
# speedup vs baseline: 1.1842x; 1.1842x over previous
"""CrossScaleAttention Trainium2 kernel.

Full (unsharded) contract: kernel(query, key, value) with shapes
  query/key/value: (4, 4096, 256) float32  ->  out (4, 4096, 256) float32

reference math:
  q = l2norm(query); k = l2norm(key)
  out = softmax((q @ k^T) * 32**-0.5) @ value

Sharding: 8 cores; core c computes batch c//2, query rows (c%2)*2048..+2048,
with that batch's full K/V resident per core (no collectives).

Per-core algorithm (all matmuls contract over the partition dim):
  - K is transposed RAW into K^T [d, keys] (PE transpose); its row norm is
    applied later as the exp's per-partition scale, so K^T production only
    depends on the DMA. Q is scaled by SCALE/||q|| before its transpose
    (a per-query scale cannot ride the exp, whose scale is per-partition).
  - row norms: sum-of-squares on DVE (one mul+reduce pair per tile group),
    rsqrt on DVE via the 0x5f3759df bit trick + 2 Newton steps (no ACT
    table switches at all -- ACT runs a single Exp table the whole kernel).
  - S^T chunks [128 keys, 512 queries] on PE (K^T chunk stationary);
    exp(scale_k * s) via ACT straight PSUM->SBUF producing P^T. No
    max-subtraction needed: scores are cosine sims * 0.177.
  - AV: out_psum[128 q, 258] += P^T_chunk.T @ [V | 1 1]; the ones columns
    accumulate the softmax denominator inside the same matmul chain
    (padded to 258: fp32r matmuls need an even moving-dim).
  - epilogue: out = out_psum[:, :256] * (1 / out_psum[:, 256]).
  - inputs arrive via a few large multi-tile DMAs (one InstDMACopy spreads
    across all 16 SDMA slots), ordered so block 0's operands land first.
"""

import sys

if "/opt/trn_rl_repo" not in sys.path:
    sys.path.insert(0, "/opt/trn_rl_repo")

import numpy as np

import concourse.bass as bass
import concourse.mybir as mybir
import concourse.tile as tile
from concourse import bacc
from concourse.bass_utils import run_bass_kernel_spmd
from concourse.masks import make_identity

F32 = mybir.dt.float32
F32R = mybir.dt.float32r
I32 = mybir.dt.int32

B, NQ_FULL, NK, D = 4, 4096, 4096, 256
N_CORES = 8
NQ = NQ_FULL * B // N_CORES  # 2048 queries per core
P = 128
DC = D // P          # 2 d-chunks
KC = NK // P         # 32 key chunks
QTI = NQ // P        # 16 q tiles
QB = 512             # queries per block
NB = NQ // QB        # 4 blocks
QT = QB // P         # 4 q-subtiles per block
VW = D + 2           # V columns padded with two 1.0 columns (even moving-dim)
NT = KC + QTI        # 48 row tiles total
SCALE = float(D // 8) ** -0.5  # head_dim**-0.5 = 32**-0.5
RSQRT_MAGIC = 0x5F3759DF

Exp = mybir.ActivationFunctionType.Exp

# natall/ssall/rinv_all positions: q0-3 -> 0..3, k0-31 -> 4..35, q4-15 -> 36..47
QPOS0, KPOS, QPOS1 = 0, 4, 36


def _build_program():
    nc = bacc.Bacc(
        "TRN2",
        target_bir_lowering=False,
        debug=False,
        enable_asserts=False,
        num_devices=N_CORES,
    )
    q_d = nc.dram_tensor("q", (NQ, D), F32, kind="ExternalInput").ap()
    k_d = nc.dram_tensor("k", (NK, D), F32, kind="ExternalInput").ap()
    v_d = nc.dram_tensor("v", (NK, D), F32, kind="ExternalInput").ap()
    o_d = nc.dram_tensor("o", (NQ, D), F32, kind="ExternalOutput").ap()

    k_re = k_d.rearrange("(i p) d -> p i d", p=P)  # [128, 32, 256]
    q_re = q_d.rearrange("(i p) d -> p i d", p=P)  # [128, 16, 256]
    v_re = v_d.rearrange("(i p) d -> p i d", p=P)  # [128, 32, 256]

    with tile.TileContext(nc) as tc:
        with (
            tc.tile_pool(name="const", bufs=1) as const_pool,
            tc.tile_pool(name="persist", bufs=1) as persist,
            tc.tile_pool(name="stage", bufs=2) as stage,
            tc.tile_pool(name="loads", bufs=4) as loads,
            tc.tile_pool(name="small", bufs=8) as small,
            tc.tile_pool(name="pt", bufs=4) as pt_pool,
            tc.tile_pool(name="outs", bufs=3) as out_pool,
            tc.tile_pool(name="ps", bufs=4, space="PSUM") as ps_pool,
            tc.tile_pool(name="avps", bufs=1, space="PSUM") as av_pool,
        ):
            ident = const_pool.tile([P, P], F32)
            make_identity(nc, ident)
            ones = const_pool.tile([P, 1], F32)
            nc.vector.memset(ones, 1.0)
            magic = const_pool.tile([P, 1], I32)
            nc.vector.memset(magic, RSQRT_MAGIC)

            # persistent operands
            kt = persist.tile([P, DC, NK], F32R)    # K^T: [d, keys] (RAW rows)
            qt = persist.tile([P, DC, NQ], F32R)    # Q^T: [d, queries] scaled
            va = persist.tile([P, KC, VW], F32R)    # [keys, d | ones ones]
            natall = persist.tile([P, NT, D], F32)  # raw rows
            ssall = persist.tile([P, NT], F32)      # row sum-of-squares
            rinv_all = persist.tile([P, NT], F32)   # (pre)scale / ||row||

            nc.vector.tensor_copy(
                va[:, :, D:VW], ones[:, :, None].to_broadcast((P, KC, 2))
            )

            # ---- input DMAs: a few multi-tile transfers, block-0 deps first
            nc.sync.dma_start(natall[:, 0:4, :], q_re[:, 0:4, :])      # q0-3
            nc.sync.dma_start(natall[:, 4:12, :], k_re[:, 0:8, :])     # k0-7
            vstg = []
            for g, (v0, v1) in enumerate(((0, 8), (8, 16), (16, 24), (24, KC))):
                vs = stage.tile([P, 8, D], F32, tag="vl", name=f"vs{g}")
                vstg.append((vs, v0, v1))
            nc.sync.dma_start(vstg[0][0], v_re[:, 0:8, :])
            nc.sync.dma_start(vstg[1][0], v_re[:, 8:16, :])
            nc.sync.dma_start(natall[:, 12:20, :], k_re[:, 8:16, :])   # k8-15
            nc.sync.dma_start(vstg[2][0], v_re[:, 16:24, :])
            nc.sync.dma_start(natall[:, 20:36, :], k_re[:, 16:KC, :])  # k16-31
            nc.sync.dma_start(vstg[3][0], v_re[:, 24:KC, :])
            nc.sync.dma_start(natall[:, 36:48, :], q_re[:, 4:QTI, :])  # q4-15

            # V copies (f32 -> f32r round): first chunk on idle ACT; the rest
            # in small per-2-tile pieces the DVE scheduler can interleave,
            # emitted below at their DMA-arrival points.
            nc.scalar.copy(va[:, 0:8, :D], vstg[0][0])

            def v_copies(g):
                vs, v0, v1 = vstg[g]
                for j in range(0, v1 - v0, 2):
                    nc.vector.tensor_copy(
                        va[:, v0 + j : v0 + j + 2, :D], vs[:, j : j + 2, :]
                    )

            # ---- row norms (all DVE; no ACT tables involved) ----
            def norms(lo, hi, q_scale):
                n = hi - lo
                sq = stage.tile([P, n, D], F32, tag="sqg", name=f"sqg{lo}")
                nat = natall[:, lo:hi, :]
                nc.vector.tensor_mul(sq, nat, nat)
                ss = ssall[:, lo:hi]
                nc.vector.tensor_reduce(
                    ss, sq, axis=mybir.AxisListType.X, op=mybir.AluOpType.add
                )
                # rsqrt: bit-trick seed + 2 Newton iterations (err ~5e-6)
                y = rinv_all[:, lo:hi]
                yi = y.bitcast(I32)
                nc.vector.tensor_scalar(
                    yi, ss.bitcast(I32), 1, None,
                    op0=mybir.AluOpType.logical_shift_right,
                )
                nc.vector.tensor_tensor(
                    yi, magic.to_broadcast((P, n)), yi, mybir.AluOpType.subtract
                )
                t = small.tile([P, n], F32, tag="nt", name=f"nt{lo}")
                for _ in range(2):
                    nc.vector.tensor_mul(t, y, y)
                    nc.vector.tensor_mul(t, t, ss)
                    nc.vector.tensor_scalar(
                        t, t, -0.5, 1.5,
                        op0=mybir.AluOpType.mult, op1=mybir.AluOpType.add,
                    )
                    nc.vector.tensor_mul(y, y, t)
                if q_scale:
                    nc.vector.tensor_scalar_mul(y, y, SCALE)

            def finish(pos, kind, idx, copy_eng):
                """PE-transpose row-tile `pos` into kt/qt column idx."""
                if kind == "q":
                    src = loads.tile([P, D], F32, tag="xn", name=f"xn{pos}")
                    nc.vector.tensor_scalar_mul(
                        src, natall[:, pos, :], rinv_all[:, pos : pos + 1]
                    )
                    dst = qt
                else:
                    src = natall[:, pos, :]
                    dst = kt
                tps = ps_pool.tile([P, QB], F32, tag="st", name=f"tp{pos}")
                for dc in range(DC):
                    nc.tensor.transpose(
                        tps[:, dc * P : (dc + 1) * P],
                        src[:, dc * P : (dc + 1) * P],
                        ident,
                    )
                csrc = tps[:, :D].rearrange("p (c n) -> p c n", c=DC)
                cdst = dst[:, :, idx * P : (idx + 1) * P]
                if copy_eng == "scalar":
                    nc.scalar.copy(cdst, csrc)
                else:
                    nc.vector.tensor_copy(cdst, csrc)

            # part 1: everything block 0's first chunks need
            norms(0, 4, True)                       # q0-3
            for i in range(4):
                finish(QPOS0 + i, "q", i, "scalar")
            norms(4, 12, False)                     # k0-7
            for i in range(8):
                finish(KPOS + i, "k", i, "scalar")
            norms(12, 20, False)                    # k8-15
            v_copies(1)                             # v8-15

            # the rest of the prologue is interleaved into block 0's chunk
            # emission so every in-order engine stream matches data arrival
            after_chunk = {
                1: [lambda: [finish(KPOS + i, "k", i, "scalar") for i in range(8, 12)]],
                3: [lambda: [finish(KPOS + i, "k", i, "scalar") for i in range(12, 16)]],
                7: [lambda: v_copies(2), lambda: norms(20, 28, False)],
                9: [lambda: [finish(KPOS + i, "k", i, "scalar") for i in range(16, 20)]],
                11: [lambda: [finish(KPOS + i, "k", i, "scalar") for i in range(20, 24)]],
                13: [lambda: norms(28, 36, False), lambda: v_copies(3)],
                15: [lambda: [finish(KPOS + i, "k", i, "scalar") for i in range(24, 28)]],
                17: [lambda: [finish(KPOS + i, "k", i, "scalar") for i in range(28, KC)]],
                19: [lambda: norms(36, 48, True)],
                21: [lambda: [finish(QPOS1 + i - 4, "q", i, "scalar") for i in range(4, 8)]],
                23: [lambda: [finish(QPOS1 + i - 4, "q", i, "scalar") for i in range(8, 12)]],
                25: [lambda: [finish(QPOS1 + i - 4, "q", i, "scalar") for i in range(12, QTI)]],
            }

            # ---- main loop ----
            for blk in range(NB):
                avs = [
                    av_pool.tile([P, VW], F32, tag=f"av{t}", name=f"av{t}_{blk}")
                    for t in range(QT)
                ]
                for kk in range(KC):
                    st = ps_pool.tile([P, QB], F32, tag="st", name=f"st{blk}_{kk}")
                    for dc in range(DC):
                        nc.tensor.matmul(
                            st,
                            lhsT=kt[:, dc, kk * P : (kk + 1) * P],
                            rhs=qt[:, dc, blk * QB : (blk + 1) * QB],
                            start=(dc == 0),
                            stop=(dc == DC - 1),
                        )
                    pt = pt_pool.tile([P, QB], F32R, tag="pt", name=f"pt{blk}_{kk}")
                    nc.scalar.activation(
                        pt, st, Exp, scale=rinv_all[:, KPOS + kk : KPOS + kk + 1]
                    )
                    for t in range(QT):
                        nc.tensor.matmul(
                            avs[t],
                            lhsT=pt[:, t * P : (t + 1) * P],
                            rhs=va[:, kk, :],
                            start=(kk == 0),
                            stop=(kk == KC - 1),
                        )
                    if blk == 0:
                        for thunk in after_chunk.get(kk, ()):
                            thunk()
                for t in range(QT):
                    rec = small.tile([P, 1], F32, tag="rec")
                    nc.vector.reciprocal(rec, avs[t][:, D : D + 1])
                    ot = out_pool.tile([P, D], F32, tag="ot")
                    nc.vector.tensor_scalar_mul(ot, avs[t][:, :D], rec)
                    row = blk * QB + t * P
                    nc.sync.dma_start(o_d[row : row + P, :], ot)

    nc.compile()
    return nc


_CACHED = {}


def _get_program():
    if "nc" not in _CACHED:
        _CACHED["nc"] = _build_program()
    return _CACHED["nc"]


def _make_in_maps(query, key, value):
    in_maps = []
    for c in range(N_CORES):
        b = c // (N_CORES // B)
        qs = (c % (N_CORES // B)) * NQ
        in_maps.append(
            {
                "q": np.ascontiguousarray(query[b, qs : qs + NQ], dtype=np.float32),
                "k": np.ascontiguousarray(key[b], dtype=np.float32),
                "v": np.ascontiguousarray(value[b], dtype=np.float32),
            }
        )
    return in_maps


def run_sharded(query, key, value, trace=False):
    """Returns (out, BassKernelResults)."""
    nc = _get_program()
    in_maps = _make_in_maps(query, key, value)
    res = run_bass_kernel_spmd(nc, in_maps, core_ids=list(range(N_CORES)), trace=trace)
    out = np.empty((B, NQ_FULL, D), dtype=np.float32)
    for c in range(N_CORES):
        b = c // (N_CORES // B)
        qs = (c % (N_CORES // B)) * NQ
        out[b, qs : qs + NQ] = res.results[c]["o"]
    return out, res


def kernel(query, key, value):
    query = np.asarray(query)
    key = np.asarray(key)
    value = np.asarray(value)
    out, _ = run_sharded(query, key, value)
    return out
